# revision 1
# baseline (speedup 1.0000x reference)
"""Trainium2 Bass kernel for DeepMultiBasisBilinearNet.

Strategy: pure data-parallel over the batch (16384/8 = 2048 rows per core).
All activations kept in transposed [D, B] layout on-device so that every
matmul contraction dim lands on SBUF partitions with zero on-device
transposes (host pre-transposes x and all weights). All matmuls run in
bf16 (fp32 PSUM accumulation). The r/l projection accumulation chains are
interleaved across PSUM banks (same-bank back-to-back accumulation costs
~200 cycles/matmul in PE drain + weight-load serialization). Block-2's
LayerNorm is never applied as tensors: the LN gain is folded into the
head weights host-side and the per-column rstd/-mu*rstd fixup is applied
to the head's 10-row output, so the final-linear head consumes the
pre-LN residual (hpre2) directly and block-2's apply chain disappears.
"""

import sys

if "/opt/trn_rl_repo" not in sys.path:
    sys.path.insert(0, "/opt/trn_rl_repo")

import ml_dtypes
import numpy as np

import concourse.bass as bass
import concourse.tile as tile
from concourse import bacc, mybir
from concourse.bass_utils import run_bass_kernel_spmd

BF = mybir.dt.bfloat16
F32 = mybir.dt.float32
E4 = mybir.dt.float8e4
AF = mybir.ActivationFunctionType
ALU = mybir.AluOpType

P = 128
B, D, H, R, OUT = 16384, 1024, 4, 1024, 10
HR = H * R                 # 4096
NCORES = 8
BC = B // NCORES           # 2048 rows per core
NB = 512                   # batch tile (matmul free dim, one PSUM bank)
DC = D // P                # 8 chunks of the model dim
JC = HR // P               # 32 chunks of the bilinear dim
EGRP = 2                   # eigen-projection dout groups (PSUM pressure)
EGS = DC // EGRP           # douts per group
LN_EPS = 1e-5


def _emit_rl(nc, pools, dram, blk, acts, fillers=None):
    """Right/left projections + interaction for one batch tile.

    fillers: optional list of zero-arg callbacks, sprinkled one per jc
    chunk into the matmul stream (used for deferred stats whose inputs
    become ready while this stream keeps the PE busy).
    """
    wp, ip, pp, psr, cst = (
        pools["wp"], pools["ip"], pools["pp"], pools["ps_rl"], pools["const"],
    )
    br_sb = cst[f"br{blk}"]
    bl_sb = cst[f"bl{blk}"]
    fillers = list(fillers or [])

    inter = []
    for jc in range(JC):
        wr_t = wp.tile([P, D], BF, tag="wrl")
        nc.sync.dma_start(out=wr_t[:], in_=dram[f"wr{blk}"][jc])
        wl_t = wp.tile([P, D], BF, tag="wrl")
        nc.sync.dma_start(out=wl_t[:], in_=dram[f"wl{blk}"][jc])

        # r and l chains INTERLEAVED: consecutive matmuls alternate PSUM
        # banks, hiding the PE drain + weight-load serialization that a
        # same-bank accumulation chain incurs.
        ps_r = psr.tile([P, NB], F32, tag="rl")
        ps_l = psr.tile([P, NB], F32, tag="rl")
        for dc in range(DC):
            nc.tensor.matmul(
                ps_r[:], wr_t[:, dc * P:(dc + 1) * P], acts[dc][:],
                start=(dc == 0), stop=(dc == DC - 1),
            )
            nc.tensor.matmul(
                ps_l[:], wl_t[:, dc * P:(dc + 1) * P], acts[dc][:],
                start=(dc == 0), stop=(dc == DC - 1),
            )
        if fillers:
            fillers.pop(0)()
        # evict right off PSUM immediately (ACT) so the bank frees fast
        tmp_r = pp.tile([P, NB], F32, tag="tmp_r")
        nc.scalar.activation(tmp_r[:], ps_r[:], AF.Identity,
                             bias=br_sb[:, jc:jc + 1])
        tmp_l = pp.tile([P, NB], F32, tag="tmp_l")
        nc.scalar.activation(tmp_l[:], ps_l[:], AF.Identity,
                             bias=bl_sb[:, jc:jc + 1])

        it = ip.tile([P, NB], BF, tag="inter")
        nc.vector.tensor_mul(it[:], tmp_r[:], tmp_l[:])
        inter.append(it)
    for f in fillers:
        f()
    return inter


def _emit_eigen(nc, pools, dram, blk, inter, acts, defer_tail):
    """Eigen projection + residual + LN statistics for one batch tile.

    Returns (hpre chunks, row_box, tail): tail is a list of emission
    callbacks for the last group's stats matmuls + the row chain (their
    hpre inputs come off the DVE eviction chain, so emitting them inline
    would stall the in-order PE queue). row_box["row"] is set once the
    row chain has been emitted.
    """
    sb, wep_p, pse, pst, cst = (
        pools["sb"], pools["wep"], pools["ps_e"], pools["ps_st"],
        pools["const"],
    )
    be_sb = cst[f"be{blk}"]
    inv_d = cst["inv_d"]       # [128, 1] bf16 filled with 1/D
    st = pst.tile([64, NB], F32, tag="st")
    hpre = []

    def stats_one(do):
        def emit():
            nc.tensor.matmul(st[0:1, :], inv_d[:, 0:1], hpre[do][:],
                             start=(do == 0), stop=(do == DC - 1))
            sq = sb.tile([P, NB], BF, tag="sq", bufs=8)
            nc.scalar.activation(sq[:], hpre[do][:], AF.Square)
            nc.tensor.matmul(st[32:33, :], inv_d[:, 0:1], sq[:],
                             start=(do == 0), stop=(do == DC - 1))
        return emit

    pending_stats = []
    for g in range(EGRP):
        ps_es = [pse.tile([P, NB], F32, tag="eig", name=f"eig{i}")
                 for i in range(EGS)]
        for jc in range(JC):
            we_t = wep_p.tile([P, EGS * P], BF, tag="wep")
            nc.sync.dma_start(out=we_t[:], in_=dram[f"we{blk}"][g, jc])
            for di in range(EGS):
                nc.tensor.matmul(
                    ps_es[di][:], we_t[:, di * P:(di + 1) * P], inter[jc][:],
                    start=(jc == 0), stop=(jc == JC - 1),
                )
            # previous group's stats, a few chunks into this group's
            # stream: their hpre evictions are done by now, no PE stall
            if jc >= 2 and pending_stats:
                pending_stats.pop(0)()
        for di in range(EGS):
            do = g * EGS + di
            # fused: hpre = (psum + be) + residual, bf16 out, one DVE op
            hp = sb.tile([P, NB], BF, tag="hpre", bufs=24)
            nc.vector.scalar_tensor_tensor(hp[:], ps_es[di][:],
                                           be_sb[:, do:do + 1], acts[do][:],
                                           op0=ALU.add, op1=ALU.add)
            hpre.append(hp)
        pending_stats = [stats_one(g * EGS + di) for di in range(EGS)]

    row_box = {}

    def tail_row():
        eps_t = cst["eps"]
        mu = sb.tile([1, NB], F32, tag="mu", bufs=2)
        nc.scalar.copy(mu[:], st[0:1, :])
        var = sb.tile([1, NB], F32, tag="var", bufs=2)
        nc.vector.scalar_tensor_tensor(var[:], mu[:], -1.0, mu[:],
                                       op0=ALU.mult, op1=ALU.mult)
        nc.vector.tensor_add(var[:], var[:], st[32:33, :])
        std = sb.tile([1, NB], F32, tag="std", bufs=2)
        nc.scalar.activation(std[:], var[:], AF.Sqrt, bias=eps_t[:])
        rf = sb.tile([1, NB], F32, tag="rf", bufs=2)
        nc.vector.reciprocal_approx_fast(out=rf[:], in_=std[:])
        row = sb.tile([1, 2 * NB], BF, tag="row", bufs=3)
        nc.scalar.copy(row[:, 0:NB], rf[:])
        nc.vector.scalar_tensor_tensor(row[:, NB:2 * NB], mu[:], -1.0,
                                       rf[:], op0=ALU.mult, op1=ALU.mult)
        row_box["row"] = row

    tail = pending_stats + [tail_row]
    if not defer_tail:
        for f in tail:
            f()
        tail = []
    return hpre, row_box, tail


def _emit_ln_bcast_bf(nc, pools, row):
    """Broadcast [a | c] across partitions (two K=1 bf16 matmuls) and
    evict to bf16 SBUF immediately so the PSUM banks free early and the
    apply runs at bf16 DVE rate."""
    sb, pse, cst = pools["sb"], pools["ps_e"], pools["const"]
    ones_r = cst["ones_r"]
    a_ps = pse.tile([P, NB], F32, tag="eig", name="a_b")
    nc.tensor.matmul(a_ps[:], ones_r[:, :], row[:, 0:NB], start=True,
                     stop=True)
    c_ps = pse.tile([P, NB], F32, tag="eig", name="c_b")
    nc.tensor.matmul(c_ps[:], ones_r[:, :], row[:, NB:2 * NB],
                     start=True, stop=True)
    a_bf = sb.tile([P, NB], BF, tag="abf", bufs=3)
    nc.scalar.copy(a_bf[:], a_ps[:])
    c_bf = sb.tile([P, NB], BF, tag="cbf", bufs=3)
    nc.scalar.copy(c_bf[:], c_ps[:])
    return a_bf, c_bf


def _emit_ln_apply(nc, pools, blk, hpre, a_bf, c_bf):
    sb, pp, cst = pools["sb"], pools["pp"], pools["const"]
    g_sb = cst[f"g{blk}"]
    bb_sb = cst[f"bb{blk}"]
    outs = []
    for do in range(DC):
        u = pp.tile([P, NB], BF, tag="u")
        nc.vector.tensor_mul(u[:], hpre[do][:], a_bf[:])
        w = pp.tile([P, NB], BF, tag="w")
        nc.vector.tensor_add(w[:], u[:], c_bf[:])
        ho = sb.tile([P, NB], BF, tag=f"h{blk}", bufs=16)
        nc.scalar.activation(ho[:], w[:], AF.Identity,
                             bias=bb_sb[:, do:do + 1],
                             scale=g_sb[:, do:do + 1])
        outs.append(ho)
    return outs


def build_program(bc=BC):
    """Build the per-core SPMD program. bc = rows per core."""
    nt = bc // NB
    nc = bacc.Bacc("TRN2", target_bir_lowering=False)

    dram = {
        "xT": nc.dram_tensor("xT", [D, bc], BF, kind="ExternalInput"),
        # wf is pre-folded with the block-2 LN gain g2 (host side)
        "wf": nc.dram_tensor("wf", [P, DC * OUT], BF, kind="ExternalInput"),
        "sf": nc.dram_tensor("sf", [OUT, 1], F32, kind="ExternalInput"),
        "tf": nc.dram_tensor("tf", [OUT, 1], F32, kind="ExternalInput"),
        "outT": nc.dram_tensor("outT", [OUT, bc], F32, kind="ExternalOutput"),
    }
    for blk in (1, 2):
        dram[f"wr{blk}"] = nc.dram_tensor(f"wr{blk}", [JC, P, D], BF,
                                          kind="ExternalInput")
        dram[f"wl{blk}"] = nc.dram_tensor(f"wl{blk}", [JC, P, D], BF,
                                          kind="ExternalInput")
        dram[f"we{blk}"] = nc.dram_tensor(f"we{blk}", [EGRP, JC, P, EGS * P],
                                          BF, kind="ExternalInput")
        for nm, cols in ((f"br{blk}", JC), (f"bl{blk}", JC), (f"be{blk}", DC),
                         (f"g{blk}", DC), (f"bb{blk}", DC)):
            dram[nm] = nc.dram_tensor(nm, [P, cols], F32, kind="ExternalInput")

    with tile.TileContext(nc) as tc:
        with (
            tc.tile_pool(name="sb", bufs=2) as sb,
            tc.tile_pool(name="wp", bufs=6) as wp,
            tc.tile_pool(name="wep", bufs=12) as wep_p,
            tc.tile_pool(name="ip", bufs=40) as ip,
            tc.tile_pool(name="pp", bufs=3) as pp,
            tc.tile_pool(name="const", bufs=1) as cstp,
            tc.tile_pool(name="ps_rl", bufs=3, space="PSUM") as ps_rl,
            tc.tile_pool(name="ps_e", bufs=4, space="PSUM") as ps_e,
            tc.tile_pool(name="ps_st", bufs=1, space="PSUM") as ps_st,
        ):
            # warmup first: memset-fed throwaway matmuls start the PE before
            # any DMA lands, lifting the HAM clock gate to 8/8 early
            wm_l = cstp.tile([P, P], BF, tag="wm_l", name="wm_l")
            nc.vector.memset(wm_l[:], 0.0)
            wm_r = cstp.tile([P, NB], BF, tag="wm_r", name="wm_r")
            nc.vector.memset(wm_r[:], 0.0)
            for i in range(16):
                wps = ps_rl.tile([P, NB], F32, tag="rl", name=f"warm{i}")
                nc.tensor.matmul(wps[:], wm_l[:], wm_r[:],
                                 start=True, stop=True)

            cst = {}
            for blk in (1, 2):
                for nm, cols in ((f"br{blk}", JC), (f"bl{blk}", JC),
                                 (f"be{blk}", DC), (f"g{blk}", DC),
                                 (f"bb{blk}", DC)):
                    cst[nm] = cstp.tile([P, cols], F32, tag=nm, name=nm)
                    nc.gpsimd.dma_start(out=cst[nm][:], in_=dram[nm][:])
            cst["inv_d"] = cstp.tile([P, 1], BF, tag="inv_d", name="inv_d")
            nc.vector.memset(cst["inv_d"][:], 1.0 / D)
            cst["ones_r"] = cstp.tile([1, P], BF, tag="ones_r", name="ones_r")
            nc.vector.memset(cst["ones_r"][:], 1.0)
            cst["eps"] = cstp.tile([1, 1], F32, tag="eps", name="eps")
            nc.vector.memset(cst["eps"][:], LN_EPS)
            cst["wf"] = cstp.tile([P, DC * OUT], BF, tag="wf", name="wf_sb")
            nc.gpsimd.dma_start(out=cst["wf"][:], in_=dram["wf"][:])
            cst["sf"] = cstp.tile([OUT, 1], F32, tag="sf", name="sf_sb")
            nc.gpsimd.dma_start(out=cst["sf"][:], in_=dram["sf"][:])
            cst["tf"] = cstp.tile([OUT, 1], F32, tag="tf", name="tf_sb")
            nc.gpsimd.dma_start(out=cst["tf"][:], in_=dram["tf"][:])

            pools = {
                "sb": sb, "wp": wp, "wep": wep_p, "ip": ip, "pp": pp,
                "const": cst, "ps_rl": ps_rl, "ps_e": ps_e, "ps_st": ps_st,
            }
            ones_r = cst["ones_r"]

            def emit_head_folded(hpre2, row, t):
                """Head on the pre-LN residual: out = a2 .* (wf_g2 @ hpre2)
                + sf (x) c2 + tf. All inputs are long-ready when this is
                emitted, so the PE never waits here."""
                # 10-partition broadcasts of a2 / c2
                a_ps = ps_e.tile([P, NB], F32, tag="eig", name="ha_b")
                nc.tensor.matmul(a_ps[0:OUT, :], ones_r[:, 0:OUT],
                                 row[:, 0:NB], start=True, stop=True)
                c_ps = ps_e.tile([P, NB], F32, tag="eig", name="hc_b")
                nc.tensor.matmul(c_ps[0:OUT, :], ones_r[:, 0:OUT],
                                 row[:, NB:2 * NB], start=True, stop=True)
                hd = ps_e.tile([P, NB], F32, tag="eig", name="hd")
                for dc in range(DC):
                    nc.tensor.matmul(
                        hd[0:OUT, :], cst["wf"][:, dc * OUT:(dc + 1) * OUT],
                        hpre2[dc][:], start=(dc == 0), stop=(dc == DC - 1),
                    )
                a_sb = sb.tile([OUT, NB], F32, tag="hab", bufs=2)
                nc.scalar.copy(a_sb[:], a_ps[0:OUT, :])
                u = sb.tile([OUT, NB], F32, tag="hu", bufs=2)
                nc.vector.tensor_mul(u[:], hd[0:OUT, :], a_sb[:])
                v = sb.tile([OUT, NB], F32, tag="hv", bufs=2)
                nc.vector.scalar_tensor_tensor(v[:], c_ps[0:OUT, :],
                                               cst["sf"][:], u[:],
                                               op0=ALU.mult, op1=ALU.add)
                out_sb = sb.tile([OUT, NB], F32, tag="osb")
                nc.scalar.activation(out_sb[:], v[:], AF.Identity,
                                     bias=cst["tf"][:])
                nc.gpsimd.dma_start(out=dram["outT"][:, t * NB:(t + 1) * NB],
                                    in_=out_sb[:])

            # pending = (hpre2, row_box, t, tail) for the tile whose block-2
            # stats/row chain + head are deferred into the next tile:
            #   - the stats matmuls + row chain interleave into the next
            #     tile's block-1 r/l stream (fillers)
            #   - the folded head is emitted between block-1's eigen and
            #     its LN broadcast, giving the PE ready work while
            #     block-1's row chain runs on DVE/ACT
            pending = None
            for t in range(nt):
                x_bf = []
                for dc in range(DC):
                    xt = sb.tile([P, NB], BF, tag="xbf", bufs=16)
                    nc.sync.dma_start(
                        out=xt[:],
                        in_=dram["xT"][dc * P:(dc + 1) * P,
                                       t * NB:(t + 1) * NB],
                    )
                    x_bf.append(xt)

                prev_tail = pending[3] if pending is not None else []
                inter1 = _emit_rl(nc, pools, dram, 1, x_bf, fillers=prev_tail)
                hpre1, row1_box, _ = _emit_eigen(nc, pools, dram, 1, inter1,
                                                 x_bf, defer_tail=False)
                if pending is not None:
                    hpre2_prev, row2_box, t_prev, _ = pending
                    emit_head_folded(hpre2_prev, row2_box["row"], t_prev)
                a_bf1, c_bf1 = _emit_ln_bcast_bf(nc, pools, row1_box["row"])
                h1 = _emit_ln_apply(nc, pools, 1, hpre1, a_bf1, c_bf1)
                inter2 = _emit_rl(nc, pools, dram, 2, h1)
                hpre2, row2_box, tail2 = _emit_eigen(nc, pools, dram, 2,
                                                     inter2, h1,
                                                     defer_tail=True)
                pending = (hpre2, row2_box, t, tail2)

            # final tile: emit its deferred tail + folded head directly
            hpre2_prev, row2_box, t_prev, tail2 = pending
            for f in tail2:
                f()
            emit_head_folded(hpre2_prev, row2_box["row"], t_prev)
    nc.compile()
    return nc


def _bf(a):
    return np.ascontiguousarray(a.astype(ml_dtypes.bfloat16))


def prep_inputs(inputs, bc=BC, ncores=NCORES):
    """Host-side shard + transpose + bf16 conversion. Returns in_maps."""
    f = {k: np.asarray(v, dtype=np.float32) for k, v in inputs.items()}

    shared = {}
    for blk in (1, 2):
        for side in ("r", "l"):
            w = f[f"w{side}{blk}"].reshape(HR, D)          # [j, d]
            panel = w.reshape(JC, P, DC, P).transpose(0, 3, 2, 1)
            shared[f"w{side}{blk}"] = _bf(panel.reshape(JC, P, D))
            shared[f"b{side}{blk}"] = np.ascontiguousarray(
                f[f"b{side}{blk}"].reshape(JC, P).T)        # [128, 32]
        weT = f[f"we{blk}"].T                               # [j, d_out]
        panel = weT.reshape(JC, P, EGRP, EGS * P).transpose(2, 0, 1, 3)
        shared[f"we{blk}"] = _bf(panel)                     # [g, jc, p, 512]
        shared[f"be{blk}"] = np.ascontiguousarray(
            f[f"be{blk}"].reshape(DC, P).T)                 # [128, 8]
        shared[f"g{blk}"] = np.ascontiguousarray(
            f[f"g{blk}"].reshape(DC, P).T)
        shared[f"bb{blk}"] = np.ascontiguousarray(
            f[f"b{blk}"].reshape(DC, P).T)
    # head: fold the block-2 LN gain into wf; per-column fixup constants
    #   out = a2 .* (wf_g2 @ hpre2) + sf (x) c2 + tf
    # with sf = wf @ g2, tf = wf @ b2 + bf  (b2 = LN bias of block 2)
    wf_g2 = f["wf"] * f["g2"][None, :]                      # [OUT, D]
    shared["wf"] = _bf(wf_g2.T.reshape(DC, P, OUT).transpose(1, 0, 2)
                       .reshape(P, DC * OUT))               # [128, 80]
    shared["sf"] = np.ascontiguousarray(
        (f["wf"] * f["g2"][None, :]).sum(axis=1).reshape(OUT, 1)
        .astype(np.float32))
    shared["tf"] = np.ascontiguousarray(
        (f["wf"] @ f["b2"] + f["bf"]).reshape(OUT, 1).astype(np.float32))

    x = f["x"]
    in_maps = []
    for c in range(ncores):
        m = dict(shared)
        m["xT"] = _bf(x[c * bc:(c + 1) * bc].T)             # [1024, bc]
        in_maps.append(m)
    return in_maps


_PROGRAM_CACHE = {}


def get_program(bc=BC):
    if bc not in _PROGRAM_CACHE:
        _PROGRAM_CACHE[bc] = build_program(bc)
    return _PROGRAM_CACHE[bc]


def kernel(**inputs):
    nc = get_program(BC)
    in_maps = prep_inputs(inputs, BC, NCORES)
    res = run_bass_kernel_spmd(nc, in_maps, core_ids=list(range(NCORES)))
    out = np.concatenate([res.results[c]["outT"] for c in range(NCORES)],
                         axis=1).T
    return np.ascontiguousarray(out.astype(np.float32))


if __name__ == "__main__":
    raise SystemExit("import kernel and call kernel(**inputs); see test.py")



# revision 4
# speedup vs baseline: 1.0224x; 1.0224x over previous
"""Trainium2 Bass kernel for DeepMultiBasisBilinearNet.

Strategy: pure data-parallel over the batch (16384/8 = 2048 rows per core).
All activations kept in transposed [D, B] layout on-device so that every
matmul contraction dim lands on SBUF partitions with zero on-device
transposes (host pre-transposes x and all weights). Matmuls run in bf16
(fp32 PSUM accumulation) except block-2's eigen projection, which only
feeds block-2's LayerNorm statistics and therefore tolerates fp8:
it runs as fp8e4 DoubleRow matmuls at 2x PE rate. The actual output path
never sees that fp8 tensor: the final head is computed exactly as
  out = a2 .* (wf_g2 @ h1 + (wf_g2 @ We2) @ inter2 + wf_g2 @ be2)
        + c2 (x) sf + tf
with Wfe2 = wf_g2 @ We2 ([10, 4096]) folded host-side, so block-2's
1024-wide eigen output is never materialized in bf16 at all. The r/l
projection accumulation chains are interleaved across PSUM banks. The
head + block-2 stats accumulate into spare partition rows of a single
PSUM bank (rows 0:10 head, 32:33 mean, 64:65 sq-mean).
"""

import sys

if "/opt/trn_rl_repo" not in sys.path:
    sys.path.insert(0, "/opt/trn_rl_repo")

import ml_dtypes
import numpy as np

import concourse.bass as bass
import concourse.tile as tile
from concourse import bacc, mybir
from concourse.bass_utils import run_bass_kernel_spmd

BF = mybir.dt.bfloat16
F32 = mybir.dt.float32
E4 = mybir.dt.float8e4
AF = mybir.ActivationFunctionType
ALU = mybir.AluOpType
PM = mybir.MatmulPerfMode

P = 128
B, D, H, R, OUT = 16384, 1024, 4, 1024, 10
HR = H * R                 # 4096
NCORES = 8
BC = B // NCORES           # 2048 rows per core
NB = 512                   # batch tile (matmul free dim, one PSUM bank)
DC = D // P                # 8 chunks of the model dim
JC = HR // P               # 32 chunks of the bilinear dim
KP = JC // 2               # 16 fp8 DoubleRow k-pairs
EGRP = 2                   # eigen-projection dout groups (PSUM pressure)
EGS = DC // EGRP           # douts per group
LN_EPS = 1e-5
WS_E2 = 1024.0             # we2 fp8 pre-scale (host); dequant on eviction


def _emit_rl(nc, pools, dram, blk, acts, fillers=None, make_fp8=False):
    """Right/left projections + interaction for one batch tile.

    fillers: optional list of zero-arg callbacks, sprinkled one per jc
    chunk into the matmul stream (used for deferred stats whose inputs
    become ready while this stream keeps the PE busy).
    make_fp8: additionally emit the interaction as fp8e4 k-pair tiles
    [P, 2*NB] for the DoubleRow eigen (block 2).
    """
    wp, ip, i8p, pp, psr, cst = (
        pools["wp"], pools["ip"], pools["i8p"], pools["pp"], pools["ps_rl"],
        pools["const"],
    )
    br_sb = cst[f"br{blk}"]
    bl_sb = cst[f"bl{blk}"]
    fillers = list(fillers or [])

    inter = []
    it8s = []
    cur8 = None
    for jc in range(JC):
        wr_t = wp.tile([P, D], BF, tag="wrl")
        nc.sync.dma_start(out=wr_t[:], in_=dram[f"wr{blk}"][jc])
        wl_t = wp.tile([P, D], BF, tag="wrl")
        nc.sync.dma_start(out=wl_t[:], in_=dram[f"wl{blk}"][jc])

        # r and l chains INTERLEAVED: consecutive matmuls alternate PSUM
        # banks, hiding the PE drain + weight-load serialization that a
        # same-bank accumulation chain incurs.
        ps_r = psr.tile([P, NB], F32, tag="rl")
        ps_l = psr.tile([P, NB], F32, tag="rl")
        for dc in range(DC):
            nc.tensor.matmul(
                ps_r[:], wr_t[:, dc * P:(dc + 1) * P], acts[dc][:],
                start=(dc == 0), stop=(dc == DC - 1),
            )
            nc.tensor.matmul(
                ps_l[:], wl_t[:, dc * P:(dc + 1) * P], acts[dc][:],
                start=(dc == 0), stop=(dc == DC - 1),
            )
        if fillers:
            fillers.pop(0)()
        # evict right off PSUM immediately (ACT) so the bank frees fast
        tmp_r = pp.tile([P, NB], BF, tag="tmp_r")
        nc.scalar.activation(tmp_r[:], ps_r[:], AF.Identity,
                             bias=br_sb[:, jc:jc + 1])
        tmp_l = pp.tile([P, NB], BF, tag="tmp_l")
        nc.scalar.activation(tmp_l[:], ps_l[:], AF.Identity,
                             bias=bl_sb[:, jc:jc + 1])

        it = ip.tile([P, NB], BF, tag="inter", bufs=32)
        nc.vector.tensor_mul(it[:], tmp_r[:], tmp_l[:])
        inter.append(it)
        if make_fp8:
            if jc % 2 == 0:
                cur8 = i8p.tile([P, 2 * NB], E4, tag="it8", bufs=16)
                it8s.append(cur8)
            # fp8 copy for the DoubleRow eigen (ACT engine, DVE stays free)
            nc.scalar.copy(cur8[:, (jc % 2) * NB:(jc % 2 + 1) * NB], it[:])
    for f in fillers:
        f()
    return inter, it8s


def _emit_eigen(nc, pools, dram, blk, inter, acts, defer_tail):
    """Eigen projection + residual + LN statistics for one batch tile
    (block 1, bf16).

    Returns (hpre chunks, row_box, tail): tail is a list of emission
    callbacks for the last group's stats matmuls + the row chain (their
    hpre inputs come off the DVE eviction chain, so emitting them inline
    would stall the in-order PE queue). row_box["row"] is set once the
    row chain has been emitted.
    """
    sb, wep_p, pse, pst, cst = (
        pools["sb"], pools["wep"], pools["ps_e"], pools["ps_st"],
        pools["const"],
    )
    be_sb = cst[f"be{blk}"]
    inv_d = cst["inv_d"]       # [128, 1] bf16 filled with 1/D
    st = pst.tile([64, NB], F32, tag="st")
    hpre = []

    def stats_one(do):
        def emit():
            nc.tensor.matmul(st[0:1, :], inv_d[:, 0:1], hpre[do][:],
                             start=(do == 0), stop=(do == DC - 1))
            sq = sb.tile([P, NB], BF, tag="sq", bufs=3)
            nc.scalar.activation(sq[:], hpre[do][:], AF.Square)
            nc.tensor.matmul(st[32:33, :], inv_d[:, 0:1], sq[:],
                             start=(do == 0), stop=(do == DC - 1))
        return emit

    pending_stats = []
    for g in range(EGRP):
        ps_es = [pse.tile([P, NB], F32, tag="eig", name=f"eig{i}")
                 for i in range(EGS)]
        for jc in range(JC):
            we_t = wep_p.tile([P, EGS * P], BF, tag="wep")
            nc.sync.dma_start(out=we_t[:], in_=dram[f"we{blk}"][g, jc])
            for di in range(EGS):
                nc.tensor.matmul(
                    ps_es[di][:], we_t[:, di * P:(di + 1) * P], inter[jc][:],
                    start=(jc == 0), stop=(jc == JC - 1),
                )
            # previous group's stats, a few chunks into this group's
            # stream: their hpre evictions are done by now, no PE stall
            if jc >= 2 and pending_stats:
                pending_stats.pop(0)()
        for di in range(EGS):
            do = g * EGS + di
            # fused: hpre = (psum + be) + residual, bf16 out, one DVE op
            hp = sb.tile([P, NB], BF, tag="hpre", bufs=10)
            nc.vector.scalar_tensor_tensor(hp[:], ps_es[di][:],
                                           be_sb[:, do:do + 1], acts[do][:],
                                           op0=ALU.add, op1=ALU.add)
            hpre.append(hp)
        pending_stats = [stats_one(g * EGS + di) for di in range(EGS)]

    row_box = {}

    def tail_row():
        eps_t = cst["eps"]
        mu = sb.tile([1, NB], F32, tag="mu", bufs=2)
        nc.scalar.copy(mu[:], st[0:1, :])
        var = sb.tile([1, NB], F32, tag="var", bufs=2)
        nc.vector.scalar_tensor_tensor(var[:], mu[:], -1.0, mu[:],
                                       op0=ALU.mult, op1=ALU.mult)
        nc.vector.tensor_add(var[:], var[:], st[32:33, :])
        std = sb.tile([1, NB], F32, tag="std", bufs=2)
        nc.scalar.activation(std[:], var[:], AF.Sqrt, bias=eps_t[:])
        rf = sb.tile([1, NB], F32, tag="rf", bufs=2)
        nc.vector.reciprocal_approx_fast(out=rf[:], in_=std[:])
        row = sb.tile([1, 2 * NB], BF, tag="row", bufs=3)
        nc.scalar.copy(row[:, 0:NB], rf[:])
        nc.vector.scalar_tensor_tensor(row[:, NB:2 * NB], mu[:], -1.0,
                                       rf[:], op0=ALU.mult, op1=ALU.mult)
        row_box["row"] = row

    tail = pending_stats + [tail_row]
    if not defer_tail:
        for f in tail:
            f()
        tail = []
    return hpre, row_box, tail


def _emit_eigen2(nc, pools, dram, inter, it8s, h1):
    """Block-2 eigen in fp8 DoubleRow (feeds LN stats only) + exact head
    accumulation.

    st bank rows: 0:OUT head accumulator, 32:33 mean, 64:65 sq-mean.
    Returns (st, row_box, tail); tail = last group's stats + row chain,
    deferred into the next tile's r/l stream.
    """
    sb, wep_p, pp, pse, pshd, cst = (
        pools["sb"], pools["wep"], pools["pp"], pools["ps_e"],
        pools["ps_hd"], pools["const"],
    )
    be_sb = cst["be2"]
    inv_d = cst["inv_d"]
    st = pshd.tile([P, NB], F32, tag="st2")
    hp2 = []

    def stats_one(do):
        def emit():
            nc.tensor.matmul(st[32:33, :], inv_d[:, 0:1], hp2[do][:],
                             start=(do == 0), stop=(do == DC - 1))
            sq = sb.tile([P, NB], BF, tag="sq2", bufs=3)
            nc.scalar.activation(sq[:], hp2[do][:], AF.Square)
            nc.tensor.matmul(st[64:65, :], inv_d[:, 0:1], sq[:],
                             start=(do == 0), stop=(do == DC - 1))
        return emit

    pending_stats = []
    for g in range(EGRP):
        ps_es = [pse.tile([P, NB], F32, tag="eig", name=f"e2g{i}")
                 for i in range(EGS)]
        for kp in range(KP):
            we_t = wep_p.tile([P, EGS * 2 * P], E4, tag="wep8", bufs=8)
            nc.sync.dma_start(out=we_t[:], in_=dram["we2q8"][g, kp])
            rhs = it8s[kp][:].rearrange("p (two n) -> p two n", two=2)
            for di in range(EGS):
                lhsT = we_t[:, di * 2 * P:(di + 1) * 2 * P].rearrange(
                    "p (two m) -> p two m", two=2)
                nc.tensor.matmul(ps_es[di][:], lhsT, rhs,
                                 start=(kp == 0), stop=(kp == KP - 1),
                                 perf_mode=PM.DoubleRow)
            if kp >= 2 and pending_stats:
                pending_stats.pop(0)()
        for di in range(EGS):
            do = g * EGS + di
            # dequant eviction: t = psum/WS_E2 + be2 (ACT), then + h1 (DVE)
            t8 = pp.tile([P, NB], BF, tag="tmp_e2")
            nc.scalar.activation(t8[:], ps_es[di][:], AF.Identity,
                                 bias=be_sb[:, do:do + 1], scale=1.0 / WS_E2)
            hp = sb.tile([P, NB], BF, tag="hp2", bufs=10)
            nc.vector.tensor_add(hp[:], t8[:], h1[do][:])
            hp2.append(hp)
        pending_stats = [stats_one(g * EGS + di) for di in range(EGS)]

    # exact head accumulation into rows 0:OUT of the same bank:
    #   hd = wf_g2 @ h1 + Wfe2 @ inter2   (all inputs long-ready)
    for dc in range(DC):
        nc.tensor.matmul(st[0:OUT, :], cst["wf"][:, dc * OUT:(dc + 1) * OUT],
                         h1[dc][:], start=(dc == 0), stop=False)
    for jc in range(JC):
        nc.tensor.matmul(st[0:OUT, :], cst["wfe"][:, jc * OUT:(jc + 1) * OUT],
                         inter[jc][:], start=False, stop=(jc == JC - 1))

    row_box = {}

    def tail_row():
        eps_t = cst["eps"]
        mu = sb.tile([1, NB], F32, tag="mu2", bufs=2)
        nc.scalar.copy(mu[:], st[32:33, :])
        var = sb.tile([1, NB], F32, tag="var2", bufs=2)
        nc.vector.scalar_tensor_tensor(var[:], mu[:], -1.0, mu[:],
                                       op0=ALU.mult, op1=ALU.mult)
        nc.vector.tensor_add(var[:], var[:], st[64:65, :])
        std = sb.tile([1, NB], F32, tag="std2", bufs=2)
        nc.scalar.activation(std[:], var[:], AF.Sqrt, bias=eps_t[:])
        rf = sb.tile([1, NB], F32, tag="rf2", bufs=2)
        nc.vector.reciprocal_approx_fast(out=rf[:], in_=std[:])
        row = sb.tile([1, 2 * NB], BF, tag="row2", bufs=3)
        nc.scalar.copy(row[:, 0:NB], rf[:])
        nc.vector.scalar_tensor_tensor(row[:, NB:2 * NB], mu[:], -1.0,
                                       rf[:], op0=ALU.mult, op1=ALU.mult)
        row_box["row"] = row

    tail = pending_stats + [tail_row]
    return st, row_box, tail


def _emit_ln_bcast_bf(nc, pools, row):
    """Broadcast [a | c] across partitions (two K=1 bf16 matmuls) and
    evict to bf16 SBUF immediately so the PSUM banks free early and the
    apply runs at bf16 DVE rate."""
    sb, pse, cst = pools["sb"], pools["ps_e"], pools["const"]
    ones_r = cst["ones_r"]
    a_ps = pse.tile([P, NB], F32, tag="eig", name="a_b")
    nc.tensor.matmul(a_ps[:], ones_r[:, :], row[:, 0:NB], start=True,
                     stop=True)
    c_ps = pse.tile([P, NB], F32, tag="eig", name="c_b")
    nc.tensor.matmul(c_ps[:], ones_r[:, :], row[:, NB:2 * NB],
                     start=True, stop=True)
    a_bf = sb.tile([P, NB], BF, tag="abf", bufs=2)
    nc.scalar.copy(a_bf[:], a_ps[:])
    c_bf = sb.tile([P, NB], BF, tag="cbf", bufs=2)
    nc.scalar.copy(c_bf[:], c_ps[:])
    return a_bf, c_bf


def _emit_ln_apply(nc, pools, blk, hpre, a_bf, c_bf):
    sb, pp, cst = pools["sb"], pools["pp"], pools["const"]
    g_sb = cst[f"g{blk}"]
    bb_sb = cst[f"bb{blk}"]
    outs = []
    for do in range(DC):
        u = pp.tile([P, NB], BF, tag="u")
        nc.vector.tensor_mul(u[:], hpre[do][:], a_bf[:])
        w = pp.tile([P, NB], BF, tag="w")
        nc.vector.tensor_add(w[:], u[:], c_bf[:])
        ho = sb.tile([P, NB], BF, tag=f"h{blk}", bufs=10)
        nc.scalar.activation(ho[:], w[:], AF.Identity,
                             bias=bb_sb[:, do:do + 1],
                             scale=g_sb[:, do:do + 1])
        outs.append(ho)
    return outs


def build_program(bc=BC):
    """Build the per-core SPMD program. bc = rows per core."""
    nt = bc // NB
    nc = bacc.Bacc("TRN2", target_bir_lowering=False)

    dram = {
        "xT": nc.dram_tensor("xT", [D, bc], BF, kind="ExternalInput"),
        # wf is pre-folded with the block-2 LN gain g2 (host side)
        "wf": nc.dram_tensor("wf", [P, DC * OUT], BF, kind="ExternalInput"),
        # wfe = (wf_g2 @ we2) panels, contraction over the bilinear dim
        "wfe": nc.dram_tensor("wfe", [P, JC * OUT], BF, kind="ExternalInput"),
        "sf": nc.dram_tensor("sf", [OUT, 1], F32, kind="ExternalInput"),
        "tf": nc.dram_tensor("tf", [OUT, 1], F32, kind="ExternalInput"),
        "behead": nc.dram_tensor("behead", [OUT, 1], F32,
                                 kind="ExternalInput"),
        "outT": nc.dram_tensor("outT", [OUT, bc], F32, kind="ExternalOutput"),
        # block-2 eigen weights, fp8e4, pre-scaled by WS_E2, DoubleRow
        # k-pair layout [g, kp, p_j, (di, two, p_d)]
        "we2q8": nc.dram_tensor("we2q8", [EGRP, KP, P, EGS * 2 * P], E4,
                                kind="ExternalInput"),
    }
    for blk in (1, 2):
        dram[f"wr{blk}"] = nc.dram_tensor(f"wr{blk}", [JC, P, D], BF,
                                          kind="ExternalInput")
        dram[f"wl{blk}"] = nc.dram_tensor(f"wl{blk}", [JC, P, D], BF,
                                          kind="ExternalInput")
        names = [(f"br{blk}", JC), (f"bl{blk}", JC), (f"be{blk}", DC)]
        if blk == 1:
            names += [(f"g{blk}", DC), (f"bb{blk}", DC)]
        for nm, cols in names:
            dram[nm] = nc.dram_tensor(nm, [P, cols], F32, kind="ExternalInput")
    dram["we1"] = nc.dram_tensor("we1", [EGRP, JC, P, EGS * P], BF,
                                 kind="ExternalInput")

    with tile.TileContext(nc) as tc:
        with (
            tc.tile_pool(name="sb", bufs=2) as sb,
            tc.tile_pool(name="wp", bufs=6) as wp,
            tc.tile_pool(name="wep", bufs=12) as wep_p,
            tc.tile_pool(name="ip", bufs=32) as ip,
            tc.tile_pool(name="i8p", bufs=16) as i8p,
            tc.tile_pool(name="pp", bufs=3) as pp,
            tc.tile_pool(name="const", bufs=1) as cstp,
            tc.tile_pool(name="ps_rl", bufs=2, space="PSUM") as ps_rl,
            tc.tile_pool(name="ps_e", bufs=4, space="PSUM") as ps_e,
            tc.tile_pool(name="ps_st", bufs=1, space="PSUM") as ps_st,
            tc.tile_pool(name="ps_hd", bufs=1, space="PSUM") as ps_hd,
        ):
            # warmup first: memset-fed throwaway matmuls start the PE before
            # any DMA lands, lifting the HAM clock gate to 8/8 early
            wm_l = cstp.tile([P, P], BF, tag="wm_l", name="wm_l")
            nc.vector.memset(wm_l[:], 0.0)
            wm_r = cstp.tile([P, NB], BF, tag="wm_r", name="wm_r")
            nc.vector.memset(wm_r[:], 0.0)
            for i in range(16):
                wps = ps_rl.tile([P, NB], F32, tag="rl", name=f"warm{i}")
                nc.tensor.matmul(wps[:], wm_l[:], wm_r[:],
                                 start=True, stop=True)

            cst = {}
            const_names = []
            for blk in (1, 2):
                const_names += [(f"br{blk}", JC), (f"bl{blk}", JC),
                                (f"be{blk}", DC)]
            const_names += [("g1", DC), ("bb1", DC)]
            for nm, cols in const_names:
                cst[nm] = cstp.tile([P, cols], F32, tag=nm, name=nm)
                nc.gpsimd.dma_start(out=cst[nm][:], in_=dram[nm][:])
            cst["inv_d"] = cstp.tile([P, 1], BF, tag="inv_d", name="inv_d")
            nc.vector.memset(cst["inv_d"][:], 1.0 / D)
            cst["ones_r"] = cstp.tile([1, P], BF, tag="ones_r", name="ones_r")
            nc.vector.memset(cst["ones_r"][:], 1.0)
            cst["eps"] = cstp.tile([1, 1], F32, tag="eps", name="eps")
            nc.vector.memset(cst["eps"][:], LN_EPS)
            cst["wf"] = cstp.tile([P, DC * OUT], BF, tag="wf", name="wf_sb")
            nc.gpsimd.dma_start(out=cst["wf"][:], in_=dram["wf"][:])
            cst["wfe"] = cstp.tile([P, JC * OUT], BF, tag="wfe", name="wfe_sb")
            nc.gpsimd.dma_start(out=cst["wfe"][:], in_=dram["wfe"][:])
            for nm in ("sf", "tf", "behead"):
                cst[nm] = cstp.tile([OUT, 1], F32, tag=nm, name=f"{nm}_sb")
                nc.gpsimd.dma_start(out=cst[nm][:], in_=dram[nm][:])

            pools = {
                "sb": sb, "wp": wp, "wep": wep_p, "ip": ip, "i8p": i8p,
                "pp": pp, "const": cst, "ps_rl": ps_rl, "ps_e": ps_e,
                "ps_st": ps_st, "ps_hd": ps_hd,
            }
            ones_r = cst["ones_r"]

            def emit_head_apply(st, row, t):
                """out = a2 .* (hd + behead) + sf (x) c2 + tf, from the
                head accumulator in st rows 0:OUT. All inputs ready when
                emitted, so the PE never waits here."""
                a_ps = ps_e.tile([P, NB], F32, tag="eig", name="ha_b")
                nc.tensor.matmul(a_ps[0:OUT, :], ones_r[:, 0:OUT],
                                 row[:, 0:NB], start=True, stop=True)
                c_ps = ps_e.tile([P, NB], F32, tag="eig", name="hc_b")
                nc.tensor.matmul(c_ps[0:OUT, :], ones_r[:, 0:OUT],
                                 row[:, NB:2 * NB], start=True, stop=True)
                hd2 = sb.tile([OUT, NB], F32, tag="hd2", bufs=1)
                nc.scalar.activation(hd2[:], st[0:OUT, :], AF.Identity,
                                     bias=cst["behead"][:])
                a_sb = sb.tile([OUT, NB], F32, tag="hab", bufs=1)
                nc.scalar.copy(a_sb[:], a_ps[0:OUT, :])
                u = sb.tile([OUT, NB], F32, tag="hu", bufs=1)
                nc.vector.tensor_mul(u[:], hd2[:], a_sb[:])
                v = sb.tile([OUT, NB], F32, tag="hv", bufs=1)
                nc.vector.scalar_tensor_tensor(v[:], c_ps[0:OUT, :],
                                               cst["sf"][:], u[:],
                                               op0=ALU.mult, op1=ALU.add)
                out_sb = sb.tile([OUT, NB], F32, tag="osb", bufs=2)
                nc.scalar.activation(out_sb[:], v[:], AF.Identity,
                                     bias=cst["tf"][:])
                nc.gpsimd.dma_start(out=dram["outT"][:, t * NB:(t + 1) * NB],
                                    in_=out_sb[:])

            # pending = (st2, row_box, t, tail) for the tile whose block-2
            # stats/row chain + head application are deferred:
            #   - the stats matmuls + row chain interleave into the next
            #     tile's block-1 r/l stream (fillers)
            #   - the head application is emitted between block-1's eigen
            #     and its LN broadcast, giving the PE ready work while
            #     block-1's row chain runs on DVE/ACT
            pending = None
            for t in range(nt):
                x_bf = []
                for dc in range(DC):
                    xt = sb.tile([P, NB], BF, tag="xbf", bufs=10)
                    nc.sync.dma_start(
                        out=xt[:],
                        in_=dram["xT"][dc * P:(dc + 1) * P,
                                       t * NB:(t + 1) * NB],
                    )
                    x_bf.append(xt)

                prev_tail = pending[3] if pending is not None else []
                inter1, _ = _emit_rl(nc, pools, dram, 1, x_bf,
                                     fillers=prev_tail)
                hpre1, row1_box, _ = _emit_eigen(nc, pools, dram, 1, inter1,
                                                 x_bf, defer_tail=False)
                if pending is not None:
                    st_prev, row2_box, t_prev, _ = pending
                    emit_head_apply(st_prev, row2_box["row"], t_prev)
                a_bf1, c_bf1 = _emit_ln_bcast_bf(nc, pools, row1_box["row"])
                h1 = _emit_ln_apply(nc, pools, 1, hpre1, a_bf1, c_bf1)
                inter2, it8s = _emit_rl(nc, pools, dram, 2, h1, make_fp8=True)
                st2, row2_box, tail2 = _emit_eigen2(nc, pools, dram, inter2,
                                                    it8s, h1)
                pending = (st2, row2_box, t, tail2)

            # final tile: emit its deferred tail + head application directly
            st_prev, row2_box, t_prev, tail2 = pending
            for f in tail2:
                f()
            emit_head_apply(st_prev, row2_box["row"], t_prev)
    nc.compile()
    return nc


def _bf(a):
    return np.ascontiguousarray(a.astype(ml_dtypes.bfloat16))


def prep_inputs(inputs, bc=BC, ncores=NCORES):
    """Host-side shard + transpose + bf16/fp8 conversion. Returns in_maps."""
    f = {k: np.asarray(v, dtype=np.float32) for k, v in inputs.items()}

    shared = {}
    for blk in (1, 2):
        for side in ("r", "l"):
            w = f[f"w{side}{blk}"].reshape(HR, D)          # [j, d]
            panel = w.reshape(JC, P, DC, P).transpose(0, 3, 2, 1)
            shared[f"w{side}{blk}"] = _bf(panel.reshape(JC, P, D))
            shared[f"b{side}{blk}"] = np.ascontiguousarray(
                f[f"b{side}{blk}"].reshape(JC, P).T)        # [128, 32]
        shared[f"be{blk}"] = np.ascontiguousarray(
            f[f"be{blk}"].reshape(DC, P).T)                 # [128, 8]
    shared["g1"] = np.ascontiguousarray(f["g1"].reshape(DC, P).T)
    shared["bb1"] = np.ascontiguousarray(f["b1"].reshape(DC, P).T)

    # block-1 eigen: bf16 panels [g, jc, p_j, (di, p_d)]
    weT = f["we1"].T                                        # [j, d_out]
    panel = weT.reshape(JC, P, EGRP, EGS * P).transpose(2, 0, 1, 3)
    shared["we1"] = _bf(panel)                              # [g, jc, p, 512]

    # block-2 eigen: fp8e4 DoubleRow panels [g, kp, p_j, (di, two, p_d)],
    # pre-scaled so weight values sit in fp8's normal range
    weT2 = f["we2"].T                                       # [4096, 1024]
    pan8 = weT2.reshape(KP, 2, P, EGRP, EGS, P).transpose(3, 0, 2, 4, 1, 5)
    pan8 = np.clip(pan8 * WS_E2, -240.0, 240.0)
    shared["we2q8"] = np.ascontiguousarray(
        pan8.reshape(EGRP, KP, P, EGS * 2 * P).astype(ml_dtypes.float8_e4m3))

    # head folding (block-2 LN never applied as tensors):
    #   out = a2 .* (wf_g2 @ h1 + Wfe2 @ inter2 + behead) + sf (x) c2 + tf
    wf64 = f["wf"].astype(np.float64)
    g2_64 = f["g2"].astype(np.float64)
    we2_64 = f["we2"].astype(np.float64)
    wf_g2 = wf64 * g2_64[None, :]                           # [OUT, D]
    shared["wf"] = _bf(wf_g2.astype(np.float32).T.reshape(DC, P, OUT)
                       .transpose(1, 0, 2).reshape(P, DC * OUT))
    wfe2 = wf_g2 @ we2_64                                   # [OUT, HR]
    shared["wfe"] = _bf(wfe2.astype(np.float32).T.reshape(JC, P, OUT)
                        .transpose(1, 0, 2).reshape(P, JC * OUT))
    shared["behead"] = np.ascontiguousarray(
        (wf_g2 @ f["be2"].astype(np.float64)).reshape(OUT, 1)
        .astype(np.float32))
    shared["sf"] = np.ascontiguousarray(
        wf_g2.sum(axis=1).reshape(OUT, 1).astype(np.float32))
    shared["tf"] = np.ascontiguousarray(
        (wf64 @ f["b2"].astype(np.float64) + f["bf"]).reshape(OUT, 1)
        .astype(np.float32))

    x = f["x"]
    in_maps = []
    for c in range(ncores):
        m = dict(shared)
        m["xT"] = _bf(x[c * bc:(c + 1) * bc].T)             # [1024, bc]
        in_maps.append(m)
    return in_maps


_PROGRAM_CACHE = {}


def get_program(bc=BC):
    if bc not in _PROGRAM_CACHE:
        _PROGRAM_CACHE[bc] = build_program(bc)
    return _PROGRAM_CACHE[bc]


def kernel(**inputs):
    nc = get_program(BC)
    in_maps = prep_inputs(inputs, BC, NCORES)
    res = run_bass_kernel_spmd(nc, in_maps, core_ids=list(range(NCORES)))
    out = np.concatenate([res.results[c]["outT"] for c in range(NCORES)],
                         axis=1).T
    return np.ascontiguousarray(out.astype(np.float32))


if __name__ == "__main__":
    raise SystemExit("import kernel and call kernel(**inputs); see test.py")
